# revision 1
# baseline (speedup 1.0000x reference)
"""Nystrom attention Trainium2 kernel (v2).

Full-input contract: kernel(Q, K, V) with shapes [4, 16, 4096, 64] fp32,
returns X [4, 16, 4096, 64] fp32.  The 64 (batch, head) pairs are sharded
8-per-core across 8 NeuronCores (SPMD, no cross-core communication); each
core processes its 8 pairs as 4 groups of 2 pairs stacked in the two
64-partition halves of the 128-partition datapath.

Host-side packing (free; outside HW exec time):
  QQ[g] = [Q_pairA | Q_pairB]  [4096, 128] bf16   (XBAR dma-transpose input)
  KK[g] likewise
  VV[g] = [128(p), 64(t: A 0:32, B 32:64), 65] bf16, col 64 = 1.0,
          vn[p, t] = V[s = t*128 + p]
  XO    = [2, 128, 2, 2, 32, 64] bf16 out, X[pair][t*128+p] = XO[h, p, gi, pr, t]

Device math per group (m = 64 landmarks, all scale factors folded into the
exp-activation immediate `scale`):
  qqt/kkt [128, 4096] <- XBAR transpose load (Q^T_A rows 0:64, Q^T_B 64:128)
  qlm/klm [128, 64]   <- segment-sum tensor_reduce over 64-col windows
  e1t = exp(SC1 * Ksum . Q^T)  (stacked [128, 4096])
  e3t = exp(SC1 * K-block . Qsum^T) per pair [128, 2048]
  k2  = rownorm(exp(SC2 * Qsum . Ksum^T)) stacked [128, 64]
  CVa = e3t^T @ [V|1] stacked accum [128, 65] -> CV rownorm
  NS  = 6 Newton-Schulz iterations on the two 64x64 halves (quadrant matmuls)
  X'  = e1t^T @ [NS@CV | 1] -> divide by last col -> bf16 store
"""

import math

import numpy as np

import concourse.bass as bass
import concourse.tile as tile
from concourse import bacc, mybir

F32 = mybir.dt.float32
BF16 = mybir.dt.bfloat16

B, H, S, D = 4, 16, 4096, 64
M = 64
SEG = S // M            # 64
NT = S // 128           # 32 s-tiles per pair
N_CORES = 8
PAIRS = (B * H) // N_CORES  # 8 pairs per core
GRPS = PAIRS // 2           # 4 groups of 2 stacked pairs
SC1 = 1.0 / (8.0 * SEG)        # s^2 / seg   = 1/512    (e1t, e3t)
SC2 = 1.0 / (8.0 * SEG * SEG)  # s^2 / seg^2 = 1/32768  (e2)

Exp = mybir.ActivationFunctionType.Exp
Alu = mybir.AluOpType
AX = mybir.AxisListType

# X' psum grouping: s-tiles per psum bank group
XPG = [7, 7, 7, 7, 4]


def _consts():
    bf = mybir.dt.np(BF16)
    i128 = np.eye(128, dtype=np.float32).astype(bf)
    ii = np.concatenate([np.eye(64), np.eye(64)], axis=0).astype(np.float32)
    sel = np.zeros((1, 256), dtype=np.float32)
    sel[0, 0:64] = 1.0       # selA: partitions 0:64
    sel[0, 192:256] = 1.0    # selB: partitions 64:128
    ones128 = np.ones((128, 1), dtype=np.float32)
    return (i128, ii.astype(bf), (-7.0 * ii).astype(bf), (15.0 * ii).astype(bf),
            (-13.0 * ii).astype(bf), sel.astype(bf), ones128.astype(bf))


def build_body(tc, ctx, qq_d, kk_d, vv_d, xo_d, stop=None):
    nc = tc.nc
    (i128_np, i64x2_np, i7_np, i15_np, iq_np, sel_np,
     ones128_np) = _consts()

    i128_dram = nc.inline_tensor(i128_np, name="i128c")
    i64x2_dram = nc.inline_tensor(i64x2_np, name="i64x2c")
    in7_dram = nc.inline_tensor(i7_np, name="in7c")
    i15_dram = nc.inline_tensor(i15_np, name="i15c")
    in13_dram = nc.inline_tensor(iq_np, name="in13c")
    sel_dram = nc.inline_tensor(sel_np, name="selc")
    ones128_dram = nc.inline_tensor(ones128_np, name="ones128c")

    cpool = ctx.enter_context(tc.tile_pool(name="consts", bufs=1))
    tpool = ctx.enter_context(tc.tile_pool(name="trans", bufs=3))
    vpool = ctx.enter_context(tc.tile_pool(name="vin", bufs=3))
    epool = ctx.enter_context(tc.tile_pool(name="exps", bufs=2))
    opool = ctx.enter_context(tc.tile_pool(name="outs", bufs=2))
    spool = ctx.enter_context(tc.tile_pool(name="smalls", bufs=3))
    gpool = ctx.enter_context(tc.tile_pool(name="persist", bufs=4))
    hpool = ctx.enter_context(tc.tile_pool(name="halves", bufs=2))
    ps_big = ctx.enter_context(tc.tile_pool(name="ps_big", bufs=2, space="PSUM"))
    ps_xp = ctx.enter_context(tc.tile_pool(name="ps_xp", bufs=2, space="PSUM"))
    ps_sm = ctx.enter_context(tc.tile_pool(name="ps_sm", bufs=2, space="PSUM"))

    i128 = cpool.tile([128, 128], BF16)
    nc.sync.dma_start(out=i128[:], in_=i128_dram[:])
    i64x2 = cpool.tile([128, 64], BF16)
    nc.sync.dma_start(out=i64x2[:], in_=i64x2_dram[:])
    in7x2 = cpool.tile([128, 64], BF16)
    nc.sync.dma_start(out=in7x2[:], in_=in7_dram[:])
    i15x2 = cpool.tile([128, 64], BF16)
    nc.sync.dma_start(out=i15x2[:], in_=i15_dram[:])
    in13x2 = cpool.tile([128, 64], BF16)
    nc.sync.dma_start(out=in13x2[:], in_=in13_dram[:])
    selc = cpool.tile([1, 256], BF16)
    nc.sync.dma_start(out=selc[:], in_=sel_dram[:])
    ones128 = cpool.tile([128, 1], BF16)
    nc.sync.dma_start(out=ones128[:], in_=ones128_dram[:])

    HALVES = ((0, 64), (64, 128))

    # ---- P0: all input DMAs, ordered by first use ----
    qqt, kkt, vnt = [None] * GRPS, [None] * GRPS, [None] * GRPS

    def load_g(g):
        qt = tpool.tile([128, S], BF16, tag="qqt", name=f"qqt_{g}")
        nc.sync.dma_start_transpose(qt[:], qq_d[g])
        qqt[g] = qt
        kt = tpool.tile([128, S], BF16, tag="kkt", name=f"kkt_{g}")
        nc.sync.dma_start_transpose(kt[:], kk_d[g])
        kkt[g] = kt

    def load_v(h):
        vt = vpool.tile([128, 2, 2 * NT, 65], BF16, tag="vn", name=f"vn_{h}")
        nc.sync.dma_start(
            out=vt[:], in_=vv_d[2 * h:2 * h + 2].rearrange("g p t d -> p g t d")
        )
        vnt[2 * h] = vt[:, 0]
        vnt[2 * h + 1] = vt[:, 1]

    load_g(0)
    load_v(0)
    load_g(1)
    load_g(2)
    load_v(1)
    load_g(3)

    st = [dict() for _ in range(GRPS)]

    # ---- P1a: segment sums via binary add-tree, all on the Pool engine ----
    def tree_sum(g, srcT, nm):
        ha = hpool.tile([128, 2048], F32, tag=f"{nm}a", name=f"{nm}a_{g}")
        nc.gpsimd.tensor_tensor(ha[:, 0:2048], srcT[:, 0::2], srcT[:, 1::2],
                                op=Alu.add)
        of = spool.tile([128, 64], F32, tag=f"{nm}f", name=f"{nm}f_{g}")
        nc.vector.tensor_reduce(
            of[:], ha[:].rearrange("p (m g) -> p m g", g=SEG // 2),
            axis=AX.X, op=Alu.add,
        )
        out = gpool.tile([128, 64], BF16, tag=nm, name=f"{nm}_{g}")
        nc.gpsimd.tensor_copy(out[:], of[:])
        return out

    def p1_reduce(g):
        s = st[g]
        s["qlm"] = tree_sum(g, qqt[g][:], "qlm")
        s["klm"] = tree_sum(g, kkt[g][:], "klm")

    # ---- P1b: kernel_2 + NS init ----
    def p1_rest(g):
        s = st[g]
        qlm, klm = s["qlm"], s["klm"]
        l2_ps = ps_sm.tile([128, 64], F32, tag="sm", name=f"l2_{g}")
        for lo, hi in HALVES:
            nc.tensor.matmul(l2_ps[lo:hi, :], qlm[lo:hi, :], klm[lo:hi, :])
        e2 = spool.tile([128, 64], F32, tag="e2")
        d2 = spool.tile([128, 1], F32, tag="d2")
        nc.scalar.activation(e2[:], l2_ps[:], Exp, scale=SC2, accum_out=d2[:])
        d2i = spool.tile([128, 1], F32, tag="d2i")
        nc.vector.reciprocal(d2i[:], d2[:])
        k2 = gpool.tile([128, 64], BF16, tag="k2", name=f"k2_{g}")
        nc.gpsimd.tensor_scalar_mul(k2[:], e2[:], d2i[:])
        k2t_ps = ps_sm.tile([128, 64], BF16, tag="sm", name=f"k2tp_{g}")
        for lo, hi in HALVES:
            nc.tensor.transpose(k2t_ps[lo:hi, :], k2[lo:hi, :], i64x2[lo:hi, :])
        k2t = gpool.tile([128, 64], BF16, tag="k2t", name=f"k2t_{g}")
        nc.vector.tensor_copy(k2t[:], k2t_ps[:])

        c_ps = ps_sm.tile([128, 1], F32, tag="sm", name=f"c_{g}")
        for lo, hi in HALVES:
            nc.tensor.matmul(c_ps[lo:hi, :], k2[lo:hi, :], ones128[lo:hi, :])
        c_bf = spool.tile([128, 1], BF16, tag="cbf")
        nc.vector.tensor_copy(c_bf[:], c_ps[:])
        ct_ps = ps_sm.tile([1, 128], BF16, tag="sm", name=f"ct_{g}")
        nc.tensor.transpose(ct_ps[:], c_bf[:], i128[:])
        mx = spool.tile([1, 2], F32, tag="mx")
        nc.vector.tensor_reduce(
            mx[:], ct_ps[:].rearrange("p (a b) -> p a b", b=64),
            axis=AX.X, op=Alu.max,
        )
        mxi = spool.tile([1, 2], F32, tag="mxi")
        nc.vector.reciprocal(mxi[:], mx[:])
        mxib = spool.tile([1, 2], BF16, tag="mxib")
        nc.vector.tensor_copy(mxib[:], mxi[:])
        scb_ps = ps_sm.tile([128, 1], F32, tag="sm", name=f"scb_{g}")
        nc.tensor.matmul(scb_ps[:], selc[:, 0:128], mxib[:, 0:1],
                         start=True, stop=False)
        nc.tensor.matmul(scb_ps[:], selc[:, 128:256], mxib[:, 1:2],
                         start=False, stop=True)
        scb = spool.tile([128, 1], F32, tag="scb")
        nc.vector.tensor_copy(scb[:], scb_ps[:])
        vc = gpool.tile([128, 64], BF16, tag="vc0", name=f"vc0_{g}")
        nc.gpsimd.tensor_scalar_mul(vc[:], k2t[:], scb[:])
        vct = gpool.tile([128, 64], BF16, tag="vct0", name=f"vct0_{g}")
        nc.gpsimd.tensor_scalar_mul(vct[:], k2[:], scb[:])
        s["k2"], s["k2t"] = k2, k2t
        s["vc"], s["vct"] = vc, vct

    # ---- one Newton-Schulz round (both pairs stacked) ----
    # V' = 0.25 V (13I - KV(15I - KV(7I - KV))); all c*I - X tiles built by
    # seeding PSUM with an identity matmul and accumulating the negated
    # product (stationaries pre-negated), so no vector-engine STT is needed.
    def ns_round(g, i):
        # t1n = KV - 7I; t2 = 15I + KV t1n; t3n = KV t2 - 13I;
        # V' = -0.25 V t3n  (signs folded into seeds and final scalar)
        s = st[g]
        k2t, vc, vct = s["k2t"], s["vc"], s["vct"]
        ba_ps = ps_sm.tile([128, 128], F32, tag="sm", name=f"ba_{g}_{i}")
        for lo, hi in HALVES:
            nc.tensor.matmul(ba_ps[lo:hi, 0:64], in7x2[lo:hi, :],
                             i64x2[lo:hi, :], start=True, stop=False)
            nc.tensor.matmul(ba_ps[lo:hi, 0:64], k2t[lo:hi, :], vc[lo:hi, :],
                             start=False, stop=True)
            nc.tensor.matmul(ba_ps[lo:hi, 64:128], vc[lo:hi, :], k2t[lo:hi, :])
        ba = spool.tile([128, 128], BF16, tag="ba")
        nc.vector.tensor_copy(ba[:], ba_ps[:])
        bn_sb, at_sb = ba[:, 0:64], ba[:, 64:128]
        d_ps = ps_sm.tile([128, 64], F32, tag="sm", name=f"d_{g}_{i}")
        for lo, hi in HALVES:
            nc.tensor.matmul(d_ps[lo:hi, :], i15x2[lo:hi, :], i64x2[lo:hi, :],
                             start=True, stop=False)
            nc.tensor.matmul(d_ps[lo:hi, :], at_sb[lo:hi, :], bn_sb[lo:hi, :],
                             start=False, stop=True)
        d_sb = spool.tile([128, 64], BF16, tag="d_sb")
        nc.vector.tensor_copy(d_sb[:], d_ps[:])
        g_ps = ps_sm.tile([128, 64], F32, tag="sm", name=f"g_{g}_{i}")
        for lo, hi in HALVES:
            nc.tensor.matmul(g_ps[lo:hi, :], in13x2[lo:hi, :], i64x2[lo:hi, :],
                             start=True, stop=False)
            nc.tensor.matmul(g_ps[lo:hi, :], at_sb[lo:hi, :], d_sb[lo:hi, :],
                             start=False, stop=True)
        g_sb = spool.tile([128, 64], BF16, tag="g_sb")
        nc.vector.tensor_copy(g_sb[:], g_ps[:])
        vv_ps = ps_sm.tile([128, 128], F32, tag="sm", name=f"vv_{g}_{i}")
        for lo, hi in HALVES:
            nc.tensor.matmul(vv_ps[lo:hi, 0:64], vct[lo:hi, :], g_sb[lo:hi, :])
            nc.tensor.matmul(vv_ps[lo:hi, 64:128], g_sb[lo:hi, :],
                             vct[lo:hi, :])
        vv = spool.tile([128, 128], BF16, tag="vv", name=f"vv_{g}_{i}")
        nc.vector.tensor_scalar_mul(vv[:], vv_ps[:], -0.25)
        s["vc"], s["vct"] = vv[:, 0:64], vv[:, 64:128]

    # ---- one X' psum group: matmuls + recip + normalize into xsb ----
    def xp_group(g, pr, ng):
        s = st[g]
        lo, hi = HALVES[pr]
        w0 = sum(XPG[:ng])
        npg = XPG[ng]
        xp_ps = ps_xp.tile([128, 7, 65], F32, tag="xp", name=f"xp_{g}_{pr}_{ng}")
        for j in range(npg):
            w = w0 + j
            nc.tensor.matmul(
                xp_ps[:, j, :],
                s["e1t"][lo:hi, 128 * w:128 * (w + 1)],
                s["m2a"][lo:hi, :],
            )
        dgi = spool.tile([128, 7], F32, tag="dgi")
        nc.vector.reciprocal(dgi[:, 0:npg], xp_ps[:, 0:npg, 64])
        bcast = (dgi[:, 0:npg].rearrange("p (a b) -> p a b", b=1)
                 .broadcast_to([128, npg, 64]))
        nc.vector.tensor_tensor(
            s["xsb"][:, pr, w0:w0 + npg, :],
            xp_ps[:, 0:npg, 0:64], bcast, op=Alu.mult,
        )

    NXP = 2 * len(XPG)  # 10 xp groups per group

    def xp_item(g, k):
        pr, ng = divmod(k, len(XPG))
        xp_group(g, pr, ng)

    def store(g):
        h, gi = divmod(g, 2)
        nc.sync.dma_start(out=xo_d[h, :, gi], in_=st[g]["xsb"])

    # ---- blocks: X' of group g-1 rides inside block g ----
    import os
    stop = stop or os.environ.get("KSTOP", "")
    feats = set((os.environ.get("KFEAT") or "ns,e1t,m2,xp,store").split(","))
    if stop == "loads":
        dbg = spool.tile([128, 64], F32, tag="dbg", name="dbg")
        for g in range(GRPS):
            nc.vector.tensor_copy(dbg[:], qqt[g][:, 0:64])
            nc.vector.tensor_copy(dbg[:], kkt[g][:, 0:64])
            nc.vector.tensor_copy(dbg[:], vnt[g][:, 0:64, 0])
        return
    p1_reduce(0)
    if stop == "tree":
        for g in range(1, GRPS):
            p1_reduce(g)
        return
    p1_rest(0)
    if stop == "p1":
        return
    for g in range(GRPS):
        s = st[g]
        if g + 1 < GRPS:
            p1_reduce(g + 1)
        qlm, klm = s["qlm"], s["klm"]
        vn2 = vnt[g]
        prev = g - 1
        xk = 0

        # e3t sections + NS rounds 0..3 + prev-group X'
        e3t = []
        for pr, (lo, hi) in enumerate(HALVES):
            et = epool.tile([128, NT * 64], BF16, tag=f"e3t{pr}",
                            name=f"e3t{pr}_{g}")
            for sec in range(2):
                l3_ps = ps_big.tile([128, 1024], F32, tag="big")
                for j in range(16):
                    w = 16 * sec + j
                    nc.tensor.matmul(
                        l3_ps[:, 64 * j:64 * (j + 1)],
                        kkt[g][lo:hi, 128 * w:128 * (w + 1)],
                        qlm[lo:hi, :],
                    )
                nc.scalar.activation(
                    et[:, 1024 * sec:1024 * (sec + 1)], l3_ps[:], Exp,
                    scale=SC1,
                )
                if "ns" in feats:
                    ns_round(g, 2 * pr + sec)
                if prev >= 0 and "xp" in feats and xk < 2 * pr + sec + 1:
                    xp_item(prev, xk)
                    xk += 1
            e3t.append(et)
        if stop == "e3t" and g == 0:
            return

        cv_ps = ps_sm.tile([128, 65], F32, tag="sm", name=f"cv_{g}")
        for pr, (lo, hi) in enumerate(HALVES):
            for t in range(NT):
                nc.tensor.matmul(
                    cv_ps[lo:hi, :],
                    e3t[pr][:, 64 * t:64 * (t + 1)],
                    vn2[:, NT * pr + t, :],
                    start=(t == 0),
                    stop=(t == NT - 1),
                )
        d3i = spool.tile([128, 1], F32, tag="d3i")
        nc.vector.reciprocal(d3i[:], cv_ps[:, 64:65])
        cv = spool.tile([128, 64], BF16, tag="cv_sb")
        nc.vector.tensor_scalar_mul(cv[:], cv_ps[:, 0:64], d3i[:])
        if prev >= 0 and "xp" in feats:
            xp_item(prev, xk)
            xk += 1
        if stop == "cva" and g == 0:
            return

        # e1t sections + NS rounds 4, 5 + prev-group X'
        e1t = epool.tile([128, S], BF16, tag="e1t", name=f"e1t_{g}")
        s["e1t"] = e1t
        for sec in range(4 if "e1t" in feats else 0):
            l1_ps = ps_big.tile([128, 1024], F32, tag="big")
            for lo, hi in HALVES:
                for u in range(2):
                    nc.tensor.matmul(
                        l1_ps[lo:hi, 512 * u:512 * (u + 1)], klm[lo:hi, :],
                        qqt[g][lo:hi, 1024 * sec + 512 * u:
                               1024 * sec + 512 * (u + 1)],
                    )
            nc.scalar.activation(
                e1t[:, 1024 * sec:1024 * (sec + 1)], l1_ps[:], Exp, scale=SC1
            )
            if sec % 2 == 1 and "ns" in feats:
                ns_round(g, 4 + sec // 2)
            if prev >= 0 and "xp" in feats:
                xp_item(prev, xk)
                xk += 1
                if sec >= 2 and xk < NXP:
                    xp_item(prev, xk)
                    xk += 1

        if prev >= 0 and "xp" in feats:
            while xk < NXP:
                xp_item(prev, xk)
                xk += 1
            if "store" in feats:
                store(prev)

        if "m2" not in feats:
            s["xsb"] = opool.tile([128, 2, NT, 64], BF16, tag="xsb",
                                  name=f"xsb_{g}")
            if g + 1 < GRPS:
                p1_rest(g + 1)
            continue
        # M2' = [NS @ CV | 1]
        vct = s["vct"]
        m2_ps = ps_sm.tile([128, 64], F32, tag="sm", name=f"m2_{g}")
        for lo, hi in HALVES:
            nc.tensor.matmul(m2_ps[lo:hi, :], vct[lo:hi, :], cv[lo:hi, :])
        m2a = spool.tile([128, 65], BF16, tag="m2a")
        nc.scalar.copy(m2a[:, 0:64], m2_ps[:])
        nc.vector.memset(m2a[:, 64:65], 1.0)
        s["m2a"] = m2a
        s["xsb"] = opool.tile([128, 2, NT, 64], BF16, tag="xsb",
                              name=f"xsb_{g}")
        if g + 1 < GRPS:
            p1_rest(g + 1)

    # ---- epilogue: X' + store of the last group ----
    if "xp" in feats:
        gl = GRPS - 1
        for k in range(NXP):
            xp_item(gl, k)
        if "store" in feats:
            store(gl)


def build_nc(reps=1):
    from contextlib import ExitStack

    nc = bacc.Bacc("TRN2", target_bir_lowering=False, debug=False)
    qq_d = nc.declare_dram_parameter("QQ", [GRPS, S, 128], BF16, isOutput=False)
    kk_d = nc.declare_dram_parameter("KK", [GRPS, S, 128], BF16, isOutput=False)
    vv_d = nc.declare_dram_parameter("VV", [GRPS, 128, 2 * NT, 65], BF16,
                                     isOutput=False)
    xo_d = nc.declare_dram_parameter("XO", [2, 128, 2, 2, NT, 64], BF16,
                                     isOutput=True)
    with tile.TileContext(nc) as tc:
        with ExitStack() as ctx:
            if reps == 1:
                build_body(tc, ctx, qq_d[:], kk_d[:], vv_d[:], xo_d[:])
            else:
                with tc.For_i(0, reps, 1):
                    build_body(tc, ctx, qq_d[:], kk_d[:], vv_d[:], xo_d[:])
    nc.finalize()
    return nc


_CACHED = {}


def pack_inputs(Q, K, V):
    """Host-side packing: bf16 cast + pair stacking + V permute."""
    bf = mybir.dt.np(BF16)
    BH = B * H
    Qf = np.asarray(Q, dtype=np.float32).reshape(BH, S, D)
    Kf = np.asarray(K, dtype=np.float32).reshape(BH, S, D)
    Vf = np.asarray(V, dtype=np.float32).reshape(BH, S, D)

    # [core, grp, pair, S, D] -> [core, grp, S, pair*D]
    QQ = np.ascontiguousarray(
        Qf.reshape(N_CORES, GRPS, 2, S, D).transpose(0, 1, 3, 2, 4)
        .reshape(N_CORES, GRPS, S, 128)
    ).astype(bf)
    KK = np.ascontiguousarray(
        Kf.reshape(N_CORES, GRPS, 2, S, D).transpose(0, 1, 3, 2, 4)
        .reshape(N_CORES, GRPS, S, 128)
    ).astype(bf)
    # [core, grp, pair, t, p, D] -> [core, grp, p, pair*t, D] (+ ones col)
    VV = np.ones((N_CORES, GRPS, 128, 2 * NT, 65), dtype=bf)
    VV[..., 0:64] = (
        Vf.reshape(N_CORES, GRPS, 2, NT, 128, D).transpose(0, 1, 4, 2, 3, 5)
        .reshape(N_CORES, GRPS, 128, 2 * NT, D)
    ).astype(bf)
    return QQ, KK, VV


def kernel(Q: np.ndarray, K: np.ndarray, V: np.ndarray) -> np.ndarray:
    from concourse.bass_utils import run_bass_kernel_spmd

    if "nc" not in _CACHED:
        _CACHED["nc"] = build_nc()
    nc = _CACHED["nc"]

    QQ, KK, VV = pack_inputs(Q, K, V)
    core_ids = list(range(N_CORES))
    in_maps = [
        {"QQ": QQ[c], "KK": KK[c], "VV": VV[c]} for c in core_ids
    ]
    res = run_bass_kernel_spmd(nc, in_maps, core_ids)
    # XO[h, p, gi, pr, t, d] -> X[pair = (2h+gi)*2+pr][t*128+p]
    out = np.empty((B * H, S, D), dtype=np.float32)
    for c in core_ids:
        xo = np.asarray(res.results[c]["XO"]).astype(np.float32)
        out[c * PAIRS:(c + 1) * PAIRS] = (
            xo.transpose(0, 2, 3, 4, 1, 5).reshape(PAIRS, S, D)
        )
    return out.reshape(B, H, S, D)

